# revision 6
# baseline (speedup 1.0000x reference)
"""nn_BinaryLinear TRN2 kernel: out = x @ sign(weight).T + sign(bias).

Full-input contract: kernel(x[8192,4096] f32, weight[4096,4096] f32(+-1),
bias[4096] f32(+-1)) -> out [8192, 4096] f32.

Sharding: batch 4-way x out-dim 2-way over 8 NeuronCores; each core computes
an independent [2048, 2048] output block (no collectives), assembled on host.
Host sharding feeds x and weight pre-transposed ([K, Bs]/[K, Os] layouts), so
the kernel needs no PE transposes at all.

Per-core design: K is split 50/50 into an fp8 half and an fp16 half.
- k 0..2047: x and W cast to fp8e4 (W is exactly +-1, lossless) and run as
  DoubleRow matmuls (256 k per instruction, ~1.8x fp16 rate).
- k 2048..4095: x cast to fp16 (2^-11 exact-ish), standard matmuls.
Both accumulate f32 into the same PSUM group; measured rel err ~1.88e-2.

W streams in n-segment-major order (all K chunks for one 512-wide output
column segment), so the first 4 m-tiles can compute segment-by-segment while
W loads - the fill phase keeps the PE ~90% busy. After W is fully resident,
the remaining m-tiles run m-major with 4 psum banks + 4 pipelined.

x m-tiles load via gpsimd casting DMAs (f32->fp8/fp16 in flight); W loads
f32 on the sync HWDGE queue and is cast by DVE (fp8 half) / ACT (fp16 half).
"""

from contextlib import ExitStack

import numpy as np

import concourse.bass as bass
import concourse.tile as tile
from concourse import bacc, mybir
from concourse.bass_utils import run_bass_kernel_spmd

P = 128
F32 = mybir.dt.float32
FP16 = mybir.dt.float16
FP8 = mybir.dt.float8e4
DR = mybir.MatmulPerfMode.DoubleRow

B, K, O = 8192, 4096, 4096
BSHARD, OSHARD = 4, 2
Bs, Os = B // BSHARD, O // OSHARD


def _build(Bs=2048, Ks=4096, Os=2048, C8=8, FILLM=6):
    KT = Ks // P              # 32 k-subtiles of 128
    KT16_0 = 2 * C8           # first fp16 subtile (fp8 covers kt 0..2*C8-1)
    NSEG = Os // 512          # 4 output column segments
    MT = Bs // P              # 16 m-tiles
    FILLM = min(FILLM, MT)

    nc = bacc.Bacc("TRN2", target_bir_lowering=False, debug=False)
    x = nc.dram_tensor("x", [Ks, Bs], F32, kind="ExternalInput").ap()
    w = nc.dram_tensor("weight", [Ks, Os], F32, kind="ExternalInput").ap()
    b = nc.dram_tensor("bias", [Os], F32, kind="ExternalInput").ap()
    out = nc.dram_tensor("out", [Bs, Os], F32, kind="ExternalOutput").ap()

    x_r = x.rearrange("(kt p) (m j) -> p kt m j", p=P, j=P)
    w8_r = w.rearrange("(c i p) (s n) -> p c i s n", p=P, i=2, n=512)
    w16_r = w.rearrange("(kt p) (s n) -> p kt s n", p=P, n=512)
    out_r = out.rearrange("(m p) o -> p m o", p=P)

    with tile.TileContext(nc) as tc, ExitStack() as ctx:
        const = ctx.enter_context(tc.tile_pool(name="const", bufs=1))
        w8p = ctx.enter_context(tc.tile_pool(name="w8", bufs=C8 * NSEG))
        w16p = ctx.enter_context(tc.tile_pool(name="w16", bufs=(KT - KT16_0) * NSEG))
        ws8p = ctx.enter_context(tc.tile_pool(name="ws8", bufs=2))
        ws16p = ctx.enter_context(tc.tile_pool(name="ws16", bufs=3))
        xsp = ctx.enter_context(tc.tile_pool(name="xs", bufs=2))
        # FILLM tiles stay live through the whole fill phase + steady prefetch
        x8p = ctx.enter_context(tc.tile_pool(name="x8", bufs=FILLM + 2))
        x16p = ctx.enter_context(tc.tile_pool(name="x16", bufs=FILLM + 2))
        ostage = ctx.enter_context(tc.tile_pool(name="ostage", bufs=4))
        psum = ctx.enter_context(tc.tile_pool(name="psum", bufs=8, space="PSUM"))

        def stage_x(m):
            # fp8 half casts in-flight on the gpsimd sw-DGE; fp16 half loads
            # f32 on the scalar HWDGE queue and casts on ACT, splitting the
            # staging bandwidth across engines.
            x8 = x8p.tile([P, KT16_0, P], FP8, tag="x8")
            nc.gpsimd.dma_start(out=x8[:], in_=x_r[:, 0:KT16_0, m, :])
            st = xsp.tile([P, KT - KT16_0, P], F32, tag="xs")
            nc.scalar.dma_start(st[:], x_r[:, KT16_0:KT, m, :])
            x16 = x16p.tile([P, KT - KT16_0, P], FP16, tag="x16")
            nc.scalar.copy(out=x16[:], in_=st[:])
            return x8, x16

        def load_w_seg(s):
            for c in range(C8):
                st = ws8p.tile([P, 2, 512], F32, tag="ws8")
                nc.sync.dma_start(st[:], w8_r[:, c, :, s, :])
                t = w8p.tile([P, 2, 512], FP8, tag="w8", name=f"w8_{c}_{s}")
                nc.vector.tensor_copy(out=t[:], in_=st[:])
                w8t[c, s] = t
            for kt in range(KT16_0, KT):
                st = ws16p.tile([P, 512], F32, tag="ws16")
                nc.sync.dma_start(st[:], w16_r[:, kt, s, :])
                t = w16p.tile([P, 512], FP16, tag="w16", name=f"w16_{kt}_{s}")
                nc.scalar.copy(out=t[:], in_=st[:])
                w16t[kt, s] = t

        def mm_group(pm, x8, x16, s):
            for c in range(C8):
                nc.tensor.matmul(
                    pm[:], x8[:, 2 * c : 2 * c + 2, :], w8t[c, s][:],
                    start=(c == 0), stop=False, perf_mode=DR,
                )
            for kt in range(KT16_0, KT):
                nc.tensor.matmul(
                    pm[:], x16[:, kt - KT16_0, :], w16t[kt, s][:],
                    start=False, stop=(kt == KT - 1),
                )

        def evict(m, s, pm):
            o32 = ostage.tile([P, 512], F32, tag="o32")
            ns = slice(s * 512, (s + 1) * 512)
            nc.vector.tensor_add(out=o32[:], in0=pm[:], in1=bias_sb[:, ns])
            nc.sync.dma_start(out_r[:, m, ns], o32[:])

        w8t, w16t = {}, {}
        fill_x = [stage_x(m) for m in range(2)]

        # bias: issued after the first x stages so the gpsimd queue head
        # starts the x8[m0] cast DMA immediately
        bias_sb = const.tile([P, Os], F32)
        nc.sync.dma_start(bias_sb[:1, :], b.rearrange("(a o) -> a o", a=1))
        nc.gpsimd.partition_broadcast(bias_sb[:], bias_sb[:1, :])

        fill_x += [stage_x(m) for m in range(2, FILLM)]

        # fill: W streams segment-major; m-tiles 0..FILLM-1 compute per segment
        for s in range(NSEG):
            load_w_seg(s)
            for m in range(FILLM):
                pm = psum.tile([P, 512], F32, tag="pm")
                mm_group(pm, fill_x[m][0], fill_x[m][1], s)
                evict(m, s, pm)

        # steady state: x streams two m-tiles ahead, W fully resident
        xs = {}
        for mp in range(FILLM, min(FILLM + 2, MT)):
            xs[mp] = stage_x(mp)
        for m in range(FILLM, MT):
            if m + 2 < MT:
                xs[m + 2] = stage_x(m + 2)
            x8, x16 = xs.pop(m)
            for s in range(NSEG):
                pm = psum.tile([P, 512], F32, tag="pm")
                mm_group(pm, x8, x16, s)
                evict(m, s, pm)

    nc.compile()
    return nc


_NC_CACHE = {}


def _get_nc():
    if "nc" not in _NC_CACHE:
        _NC_CACHE["nc"] = _build(Bs=Bs, Ks=K, Os=Os)
    return _NC_CACHE["nc"]


def _shard_inputs(x, weight, bias):
    xT_parts = [
        np.ascontiguousarray(x[i * Bs : (i + 1) * Bs].T) for i in range(BSHARD)
    ]
    wT_parts = [
        np.ascontiguousarray(weight[j * Os : (j + 1) * Os].T) for j in range(OSHARD)
    ]
    in_maps = []
    for c in range(8):
        bi, oj = divmod(c, OSHARD)
        in_maps.append(
            {
                "x": xT_parts[bi],
                "weight": wT_parts[oj],
                "bias": np.ascontiguousarray(bias[oj * Os : (oj + 1) * Os]),
            }
        )
    return in_maps


def kernel(x, weight, bias, _trace=False, **_kw):
    x = np.asarray(x, dtype=np.float32)
    weight = np.asarray(weight, dtype=np.float32)
    bias = np.asarray(bias, dtype=np.float32)

    nc = _get_nc()
    in_maps = _shard_inputs(x, weight, bias)
    res = run_bass_kernel_spmd(nc, in_maps, core_ids=list(range(8)), trace=_trace)

    out = np.empty((B, O), dtype=np.float32)
    for c in range(8):
        bi, oj = divmod(c, OSHARD)
        out[bi * Bs : (bi + 1) * Bs, oj * Os : (oj + 1) * Os] = res.results[c]["out"]
    if _trace:
        kernel.last_results = res
    return out


# revision 14
# speedup vs baseline: 1.2159x; 1.2159x over previous
"""nn_BinaryLinear TRN2 kernel: out = x @ sign(weight).T + sign(bias).

Full-input contract: kernel(x[8192,4096] f32, weight[4096,4096] f32(+-1),
bias[4096] f32(+-1)) -> out [8192, 4096] f32.

Sharding: batch 4-way x out-dim 2-way over 8 NeuronCores; each core computes
an independent [2048, 2048] output block (no collectives), assembled on host.
Host sharding feeds x and weight pre-transposed ([K, Bs]/[K, Os] layouts), so
the kernel needs no PE transposes at all.

Per-core design: K is split 50/50 into an fp8 half and an fp16 half.
- k 0..2047: x and W cast to fp8e4 (W is exactly +-1, lossless) and run as
  DoubleRow matmuls (256 k per instruction, ~1.8x fp16 rate).
- k 2048..4095: x cast to fp16 (2^-11 exact-ish), standard matmuls.
Both accumulate f32 into the same PSUM group; measured rel err ~1.88e-2.

W streams in n-segment-major order (all K chunks for one 512-wide output
column segment), so the first 4 m-tiles can compute segment-by-segment while
W loads - the fill phase keeps the PE ~90% busy. After W is fully resident,
the remaining m-tiles run m-major with 4 psum banks + 4 pipelined.

x m-tiles load via gpsimd casting DMAs (f32->fp8/fp16 in flight); W loads
f32 on the sync HWDGE queue and is cast by DVE (fp8 half) / ACT (fp16 half).
"""

from contextlib import ExitStack

import numpy as np

import concourse.bass as bass
import concourse.tile as tile
from concourse import bacc, mybir
from concourse.bass_utils import run_bass_kernel_spmd

P = 128
F32 = mybir.dt.float32
FP16 = mybir.dt.float16
FP8 = mybir.dt.float8e4
DR = mybir.MatmulPerfMode.DoubleRow

B, K, O = 8192, 4096, 4096
BSHARD, OSHARD = 4, 2
Bs, Os = B // BSHARD, O // OSHARD


def _build(Bs=2048, Ks=4096, Os=2048, C8=8, FILLM=6):
    KT = Ks // P              # 32 k-subtiles of 128
    KT16_0 = 2 * C8           # first fp16 subtile (fp8 covers kt 0..2*C8-1)
    NSEG = Os // 512          # 4 output column segments
    MT = Bs // P              # 16 m-tiles
    FILLM = min(FILLM, MT)

    nc = bacc.Bacc("TRN2", target_bir_lowering=False, debug=False)
    x = nc.dram_tensor("x", [Ks, Bs], F32, kind="ExternalInput").ap()
    w = nc.dram_tensor("weight", [Ks, Os], F32, kind="ExternalInput").ap()
    b = nc.dram_tensor("bias", [Os], F32, kind="ExternalInput").ap()
    out = nc.dram_tensor("out", [Bs, Os], F32, kind="ExternalOutput").ap()

    x_r = x.rearrange("(kt p) (m j) -> p kt m j", p=P, j=P)
    w8_r = w.rearrange("(c i p) (s n) -> p c i s n", p=P, i=2, n=512)
    w16_r = w.rearrange("(kt p) (s n) -> p kt s n", p=P, n=512)
    out_r = out.rearrange("(m p) o -> p m o", p=P)

    with tile.TileContext(nc) as tc, ExitStack() as ctx:
        const = ctx.enter_context(tc.tile_pool(name="const", bufs=1))
        w8p = ctx.enter_context(tc.tile_pool(name="w8", bufs=C8 * NSEG))
        w16p = ctx.enter_context(tc.tile_pool(name="w16", bufs=(KT - KT16_0) * NSEG))
        ws8p = ctx.enter_context(tc.tile_pool(name="ws8", bufs=3))
        ws16p = ctx.enter_context(tc.tile_pool(name="ws16", bufs=4))
        # FILLM tiles stay live through the whole fill phase + steady prefetch
        x8p = ctx.enter_context(tc.tile_pool(name="x8", bufs=FILLM + 2))
        x16p = ctx.enter_context(tc.tile_pool(name="x16", bufs=FILLM + 2))
        ostage = ctx.enter_context(tc.tile_pool(name="ostage", bufs=4))
        psum = ctx.enter_context(tc.tile_pool(name="psum", bufs=8, space="PSUM"))

        def stage_x(m):
            # both halves cast in-flight on the gpsimd sw-DGE (f32 -> fp8/fp16)
            x8 = x8p.tile([P, KT16_0, P], FP8, tag="x8")
            nc.gpsimd.dma_start(out=x8[:], in_=x_r[:, 0:KT16_0, m, :])
            x16 = x16p.tile([P, KT - KT16_0, P], FP16, tag="x16")
            nc.gpsimd.dma_start(out=x16[:], in_=x_r[:, KT16_0:KT, m, :])
            return x8, x16

        def load_w_seg(s):
            # all W casts on ACT: the DVE carries only evict adds, so neither
            # engine's FIFO ever blocks the other's upstream work
            for c in range(C8):
                st = ws8p.tile([P, 2, 512], F32, tag="ws8")
                nc.sync.dma_start(st[:], w8_r[:, c, :, s, :])
                t = w8p.tile([P, 2, 512], FP8, tag="w8", name=f"w8_{c}_{s}")
                nc.scalar.copy(out=t[:], in_=st[:])
                w8t[c, s] = t
            for kt in range(KT16_0, KT):
                st = ws16p.tile([P, 512], F32, tag="ws16")
                nc.sync.dma_start(st[:], w16_r[:, kt, s, :])
                t = w16p.tile([P, 512], FP16, tag="w16", name=f"w16_{kt}_{s}")
                nc.scalar.copy(out=t[:], in_=st[:])
                w16t[kt, s] = t

        def mm_group(pm, x8, x16, s):
            for c in range(C8):
                nc.tensor.matmul(
                    pm[:], x8[:, 2 * c : 2 * c + 2, :], w8t[c, s][:],
                    start=(c == 0), stop=False, perf_mode=DR,
                )
            for kt in range(KT16_0, KT):
                nc.tensor.matmul(
                    pm[:], x16[:, kt - KT16_0, :], w16t[kt, s][:],
                    start=False, stop=(kt == KT - 1),
                )

        def evict(m, s, pm):
            # out writes go on the scalar HWDGE queue: the sync queue stays
            # dedicated to the W input stream
            o32 = ostage.tile([P, 512], F32, tag="o32")
            ns = slice(s * 512, (s + 1) * 512)
            nc.vector.tensor_add(out=o32[:], in0=pm[:], in1=bias_sb[:, ns])
            nc.scalar.dma_start(out_r[:, m, ns], o32[:])

        w8t, w16t = {}, {}
        fill_x = [stage_x(m) for m in range(FILLM)]
        load_w_seg(0)

        # bias: issued after segment 0's W loads so they head the sync queue
        bias_sb = const.tile([P, Os], F32)
        nc.sync.dma_start(bias_sb[:1, :], b.rearrange("(a o) -> a o", a=1))
        nc.gpsimd.partition_broadcast(bias_sb[:], bias_sb[:1, :])

        # fill: W streams segment-major. MMs are emitted chunk-outer so the
        # in-order PE consumes each W tile across all FILLM m-tiles as it
        # arrives, instead of m0's whole group head-blocking the queue.
        for s in range(NSEG):
            pms = [
                psum.tile([P, 512], F32, tag="pm", name=f"pm_f{s}_{i}")
                for i in range(FILLM)
            ]
            for c in range(C8):
                for m in range(FILLM):
                    nc.tensor.matmul(
                        pms[m][:], fill_x[m][0][:, 2 * c : 2 * c + 2, :], w8t[c, s][:],
                        start=(c == 0), stop=False, perf_mode=DR,
                    )
            for kt in range(KT16_0, KT):
                for m in range(FILLM):
                    nc.tensor.matmul(
                        pms[m][:], fill_x[m][1][:, kt - KT16_0, :], w16t[kt, s][:],
                        start=False, stop=(kt == KT - 1),
                    )
            # next segment's loads+casts are emitted before this segment's
            # evicts: ACT runs the casts while the PE still chews on segment s,
            # and only then blocks on the out-DMA dispatches
            if s + 1 < NSEG:
                load_w_seg(s + 1)
            for m in range(FILLM):
                evict(m, s, pms[m])

        # steady state: x streams two m-tiles ahead, W fully resident
        xs = {}
        for mp in range(FILLM, min(FILLM + 2, MT)):
            xs[mp] = stage_x(mp)
        for m in range(FILLM, MT):
            if m + 2 < MT:
                xs[m + 2] = stage_x(m + 2)
            x8, x16 = xs.pop(m)
            for s in range(NSEG):
                pm = psum.tile([P, 512], F32, tag="pm")
                mm_group(pm, x8, x16, s)
                evict(m, s, pm)

    nc.compile()
    return nc


_NC_CACHE = {}


def _get_nc():
    if "nc" not in _NC_CACHE:
        _NC_CACHE["nc"] = _build(Bs=Bs, Ks=K, Os=Os)
    return _NC_CACHE["nc"]


def _shard_inputs(x, weight, bias):
    xT_parts = [
        np.ascontiguousarray(x[i * Bs : (i + 1) * Bs].T) for i in range(BSHARD)
    ]
    wT_parts = [
        np.ascontiguousarray(weight[j * Os : (j + 1) * Os].T) for j in range(OSHARD)
    ]
    in_maps = []
    for c in range(8):
        bi, oj = divmod(c, OSHARD)
        in_maps.append(
            {
                "x": xT_parts[bi],
                "weight": wT_parts[oj],
                "bias": np.ascontiguousarray(bias[oj * Os : (oj + 1) * Os]),
            }
        )
    return in_maps


def kernel(x, weight, bias, _trace=False, **_kw):
    x = np.asarray(x, dtype=np.float32)
    weight = np.asarray(weight, dtype=np.float32)
    bias = np.asarray(bias, dtype=np.float32)

    nc = _get_nc()
    in_maps = _shard_inputs(x, weight, bias)
    res = run_bass_kernel_spmd(nc, in_maps, core_ids=list(range(8)), trace=_trace)

    out = np.empty((B, O), dtype=np.float32)
    for c in range(8):
        bi, oj = divmod(c, OSHARD)
        out[bi * Bs : (bi + 1) * Bs, oj * Os : (oj + 1) * Os] = res.results[c]["out"]
    if _trace:
        kernel.last_results = res
    return out
